# revision 16
# baseline (speedup 1.0000x reference)
"""Trainium2 Bass kernel for nn_BinarizedLinearBlock.

Computes y = clip(BatchNorm1d(x) @ sign(W)^T, -1, 1) for
x [8192, 2048] f32, W [2048, 2048] f32, gamma/beta [2048] f32.

Strategy (8 NeuronCores, data-parallel over batch):
  - Each core gets a batch shard x_j [1024, 2048] and the full weight.
  - BN statistics need the whole batch: each core computes partial
    (sum, sumsq) per feature from its shard, a 16 KB AllReduce combines
    them, then each core normalizes its shard locally.
  - x is transposed on the PE (128x128 tiles via identity matmul) into
    xT [IN_p, B_free] layout so the contraction dim (IN) is on
    partitions; normalization is then a per-partition fused
    multiply-add on DVE.
  - W is cast to bf16 during the SWDGE load, transposed on PE as bf16,
    and binarized (ACT Sign) during the PSUM->SBUF eviction.  sign(w)
    is exactly representable in bf16.
  - Main matmul: lhsT = xT tile viewed as float32r (FP22 - full-rate
    on PE for moving dim >= 256), rhs = sign(W)^T in bf16.  PSUM
    accumulates fp32 over the 16 k-tiles; eviction fuses the hardtanh
    clip via a single DVE tensor_scalar (min 1, max -1).
"""

import sys

sys.path.insert(0, "/opt/trn_rl_repo")

import numpy as np
import ml_dtypes

import concourse.bass as bass
import concourse.bacc as bacc
import concourse.mybir as mybir
import concourse.tile as tile
from concourse.bass_utils import run_bass_kernel_spmd

F32 = mybir.dt.float32
F32R = mybir.dt.float32r
F16 = mybir.dt.float16
BF16 = mybir.dt.bfloat16
ALU = mybir.AluOpType
AFT = mybir.ActivationFunctionType

B, IN, OUT = 8192, 2048, 2048
NCORES = 8
BSH = B // NCORES          # 1024 batch rows per core
KB = BSH // 128            # 8 batch tiles per core
KI = IN // 128             # 16 contraction (input-feature) tiles
KO = OUT // 128            # 16 output-feature tiles
BN_EPS = 1e-5


def build_kernel_body(tc, y_d, x_d, w_d, gam_d, bet_d, idf_d, idb_d):
    nc = tc.nc

    consts = tc.tile_pool(name="consts", bufs=1)
    persist = tc.tile_pool(name="persist", bufs=1)
    xnat_pool = tc.tile_pool(name="xnat", bufs=2)
    wstg_pool = tc.tile_pool(name="wstg", bufs=2)
    ysb_pool = tc.tile_pool(name="ysb", bufs=3)
    tpsum = tc.tile_pool(name="tpsum", bufs=2, space="PSUM")
    ypsum = tc.tile_pool(name="ypsum", bufs=2, space="PSUM")
    dram = tc.tile_pool(name="dram", bufs=1, space="DRAM")

    ctxs = [consts, persist, xnat_pool, wstg_pool, ysb_pool, tpsum, ypsum, dram]
    entered = [c.__enter__() for c in ctxs]
    (consts, persist, xnat_pool, wstg_pool, ysb_pool, tpsum, ypsum, dram) = entered

    # ---- constants -------------------------------------------------
    ident_f = consts.tile([128, 128], F16)
    ident_b = consts.tile([128, 128], BF16)
    gamma_sb = consts.tile([128, KI], F32)
    beta_sb = consts.tile([128, KI], F32)
    zero_col = consts.tile([128, 1], F32)
    eps_col = consts.tile([128, 1], F32)
    nc.vector.memset(zero_col[:], 0.0)
    nc.vector.memset(eps_col[:], BN_EPS)
    nc.sync.dma_start(ident_f[:], idf_d[:, :])
    nc.sync.dma_start(ident_b[:], idb_d[:, :])
    nc.sync.dma_start(gamma_sb[:], gam_d[:, :])
    nc.sync.dma_start(beta_sb[:], bet_d[:, :])

    # ---- persistent SBUF tensors ----------------------------------
    xT3 = persist.tile([128, KI, BSH], F16)       # x^T, later xn^T in place
    wbT3 = persist.tile([128, KI, OUT], F16)      # sign(W)^T, exact +-1

    # ---- Phase X: load x (cast fp16), transpose on PE, evict on DVE
    # Transposes batched 4-per-PSUM-bank so the PE pipelines them and
    # evictions are one [128, 4, 128] DVE op.  bn_stats runs per
    # (b, t-group) right after each eviction, so only the last batch's
    # stats sit on the critical path to the AllReduce.
    bnst = persist.tile([128, KI, KB, 6], F32)
    bnag = persist.tile([128, KI, 2], F32)
    for b in range(KB):
        xnat = xnat_pool.tile([128, IN], F16)
        nc.gpsimd.dma_start(xnat[:], x_d[b * 128:(b + 1) * 128, :])
        for tg in range(KI // 4):
            t = tg * 4
            ps = tpsum.tile([128, 4, 128], F16, tag="xT")
            for j in range(4):
                nc.tensor.transpose(
                    ps[:, j, :], xnat[:, (t + j) * 128:(t + j + 1) * 128], ident_f[:]
                )
            nc.vector.tensor_copy(xT3[:, t:t + 4, b * 128:(b + 1) * 128], ps[:])
            for j in range(4):
                # walrus requires bn_stats output = exactly 6 elems/partition
                nc.vector.bn_stats(
                    bnst[:, t + j, b, :], xT3[:, t + j, b * 128:(b + 1) * 128]
                )
    for t in range(KI):
        nc.vector.bn_aggr(bnag[:, t, :], bnst[:, t, :, :])

    # local sums for the AllReduce: s1 = mean * BSH ; s2 = (var + mean^2) * BSH
    stats = persist.tile([128, 2 * KI], F32)
    means = bnag[:, :, 0]
    vars_ = bnag[:, :, 1]
    msq = persist.tile([128, KI], F32)
    nc.vector.tensor_scalar(stats[:, 0:KI], means, float(BSH), None, op0=ALU.mult)
    nc.vector.tensor_tensor(msq[:], means, means, op=ALU.mult)
    nc.vector.tensor_tensor(msq[:], vars_, msq[:], op=ALU.add)
    nc.vector.tensor_scalar(stats[:, KI:2 * KI], msq[:], float(BSH), None, op0=ALU.mult)

    # ---- Phase R: AllReduce over the 8 cores ----------------------
    # Bounce DMAs ride the HWDGE (sync) queue so they are not stuck
    # behind the 16 MiB weight load on the SWDGE rings.
    cc_in = dram.tile([128, 2 * KI], F32)
    cc_out = dram.tile([128, 2 * KI], F32)
    nc.sync.dma_start(cc_in[:], stats[:])
    nc.gpsimd.collective_compute(
        "AllReduce",
        ALU.add,
        replica_groups=[list(range(NCORES))],
        ins=[cc_in[:].opt()],
        outs=[cc_out[:].opt()],
    )
    gstats = persist.tile([128, 2 * KI], F32)
    nc.sync.dma_start(gstats[:], cc_out[:])

    # ---- Phase W: load W (cast to bf16), transpose on PE, sign ----
    # One SWDGE DMA per k-tile: W[:, k*128:(k+1)*128] -> [128, KO, 128]
    # (partition = within-o-block row, free = (o-block, i-col)).
    # SWDGE cast fp32 -> bf16 keeps the fp32 exponent range, so sign()
    # is never corrupted by underflow-to-zero.
    for t in range(KI):
        wstg = wstg_pool.tile([128, KO, 128], BF16)
        nc.gpsimd.dma_start(
            wstg[:],
            w_d[:, t * 128:(t + 1) * 128].rearrange("(g p) c -> p g c", p=128),
        )
        for og in range(KO // 4):
            o = og * 4
            ps = tpsum.tile([128, 4, 128], BF16, tag="wT")
            for j in range(4):
                nc.tensor.transpose(ps[:, j, :], wstg[:, o + j, :], ident_b[:])
            nc.scalar.sign(
                wbT3[:, t, o * 128:(o + 4) * 128].rearrange("p (g c) -> p g c", g=4),
                ps[:],
                bias=zero_col[:],
            )

    # ---- Phase N: compute a, c and normalize x^T in place ---------
    # a = gamma * rsqrt(var + eps);  c = beta - mean * a
    meang = persist.tile([128, KI], F32)
    ex2g = persist.tile([128, KI], F32)
    varg = persist.tile([128, KI], F32)
    stdg = persist.tile([128, KI], F32)
    invg = persist.tile([128, KI], F32)
    a_sc = persist.tile([128, KI], F32)
    c_sc = persist.tile([128, KI], F32)
    nc.vector.tensor_scalar(meang[:], gstats[:, 0:KI], 1.0 / B, None, op0=ALU.mult)
    nc.vector.tensor_scalar(ex2g[:], gstats[:, KI:2 * KI], 1.0 / B, None, op0=ALU.mult)
    nc.vector.tensor_tensor(varg[:], meang[:], meang[:], op=ALU.mult)
    nc.vector.tensor_tensor(varg[:], ex2g[:], varg[:], op=ALU.subtract)
    nc.scalar.activation(stdg[:], varg[:], AFT.Sqrt, bias=eps_col[:])
    nc.vector.reciprocal(invg[:], stdg[:])
    nc.vector.tensor_tensor(a_sc[:], gamma_sb[:], invg[:], op=ALU.mult)
    nc.vector.tensor_tensor(c_sc[:], meang[:], a_sc[:], op=ALU.mult)
    nc.vector.tensor_tensor(c_sc[:], beta_sb[:], c_sc[:], op=ALU.subtract)

    for t in range(KI):
        nc.vector.tensor_scalar(
            xT3[:, t, :], xT3[:, t, :],
            a_sc[:, t:t + 1], c_sc[:, t:t + 1],
            op0=ALU.mult, op1=ALU.add,
        )

    # ---- Phase M: main matmul + fused clip eviction ---------------
    for b in range(KB):
        for h in range(2):
            yp = ypsum.tile([128, 1024], F32)
            for t in range(KI):
                lhs = xT3[:, t, b * 128:(b + 1) * 128]
                for n2 in range(2):
                    nc.tensor.matmul(
                        yp[:, n2 * 512:(n2 + 1) * 512],
                        lhs,
                        wbT3[:, t, h * 1024 + n2 * 512: h * 1024 + (n2 + 1) * 512],
                        start=(t == 0),
                        stop=(t == KI - 1),
                    )
            ysb = ysb_pool.tile([128, 1024], F32)
            nc.vector.tensor_scalar(
                ysb[:], yp[:], 1.0, -1.0, op0=ALU.min, op1=ALU.max
            )
            nc.sync.dma_start(
                y_d[b * 128:(b + 1) * 128, h * 1024:(h + 1) * 1024], ysb[:]
            )

    for c in reversed(ctxs):
        c.__exit__(None, None, None)


def build_program():
    nc = bacc.Bacc(
        "TRN2",
        target_bir_lowering=False,
        debug=False,
        num_devices=NCORES,
    )
    x_d = nc.dram_tensor("x", [BSH, IN], F32, kind="ExternalInput")
    w_d = nc.dram_tensor("weight", [OUT, IN], F32, kind="ExternalInput")
    gam_d = nc.dram_tensor("gamma_blk", [128, KI], F32, kind="ExternalInput")
    bet_d = nc.dram_tensor("beta_blk", [128, KI], F32, kind="ExternalInput")
    idf_d = nc.dram_tensor("ident_f16", [128, 128], F16, kind="ExternalInput")
    idb_d = nc.dram_tensor("ident_bf16", [128, 128], BF16, kind="ExternalInput")
    y_d = nc.dram_tensor("y", [BSH, OUT], F32, kind="ExternalOutput")

    with tile.TileContext(nc) as tc:
        build_kernel_body(
            tc, y_d[:, :], x_d[:, :], w_d[:, :], gam_d[:, :], bet_d[:, :],
            idf_d[:, :], idb_d[:, :],
        )
    nc.compile()
    return nc


_CACHE = {}


def _get_program():
    if "nc" not in _CACHE:
        _CACHE["nc"] = build_program()
    return _CACHE["nc"]


def make_in_maps(x, weight, gamma, beta):
    x = np.ascontiguousarray(np.asarray(x, dtype=np.float32))
    weight = np.ascontiguousarray(np.asarray(weight, dtype=np.float32))
    gamma = np.asarray(gamma, dtype=np.float32)
    beta = np.asarray(beta, dtype=np.float32)
    gamma_blk = np.ascontiguousarray(gamma.reshape(KI, 128).T)
    beta_blk = np.ascontiguousarray(beta.reshape(KI, 128).T)
    ident_f = np.eye(128, dtype=np.float16)
    ident_b = np.eye(128, dtype=ml_dtypes.bfloat16)
    in_maps = []
    for j in range(NCORES):
        in_maps.append({
            "x": np.ascontiguousarray(x[j * BSH:(j + 1) * BSH]),
            "weight": weight,
            "gamma_blk": gamma_blk,
            "beta_blk": beta_blk,
            "ident_f16": ident_f,
            "ident_bf16": ident_b,
        })
    return in_maps


def run(x, weight, gamma, beta, **spmd_kwargs):
    """Run on hardware; returns (y_full, BassKernelResults)."""
    nc = _get_program()
    in_maps = make_in_maps(x, weight, gamma, beta)
    res = run_bass_kernel_spmd(nc, in_maps, core_ids=list(range(NCORES)), **spmd_kwargs)
    y = np.concatenate([r["y"] for r in res.results], axis=0)
    return np.asarray(y, dtype=np.float32), res


def run_traced(x, weight, gamma, beta, profile_dir=None):
    """Run with NTFF capture via the axon sidechannel; returns
    (y_full, per_core_exec_ns, profile_dir)."""
    import ctypes, glob, tempfile
    from concourse import bass2jax
    import gauge.profiler
    from concourse._compat import FishPath

    nc = _get_program()
    in_maps = make_in_maps(x, weight, gamma, beta)

    lib = ctypes.CDLL("/opt/axon/libaxon_pjrt.so")
    lib.axon_start_nrt_profile.argtypes = [
        ctypes.POINTER(ctypes.c_int64), ctypes.c_size_t]
    lib.axon_start_nrt_profile.restype = ctypes.c_int64
    lib.axon_stop_nrt_profile.argtypes = [ctypes.c_char_p]
    lib.axon_stop_nrt_profile.restype = ctypes.c_int64

    if profile_dir is None:
        profile_dir = tempfile.mkdtemp(prefix="ntff_")
    rc = lib.axon_start_nrt_profile(None, 0)
    assert rc == 0, f"axon_start_nrt_profile rc={rc}"
    try:
        results = bass2jax.run_bass_via_pjrt(nc, in_maps, n_cores=NCORES)
    finally:
        n = lib.axon_stop_nrt_profile(profile_dir.encode())
    y = np.concatenate([r["y"] for r in results], axis=0)
    if n <= 0:
        return np.asarray(y, dtype=np.float32), None, profile_dir

    profile = gauge.profiler.Profile(
        profile_path=FishPath(profile_dir),
        kernel_dev_mode=True,
        profile_on_exit=False,
        bass_kernel=nc.m,
        offline_processing=True,
        fname="*_body*",
    )
    perfetto_results = profile.to_perfetto(model_index=tuple(range(NCORES)))
    exec_ns = {}
    for i, pr in enumerate(perfetto_results or []):
        exec_ns[i] = pr.exec_time_ns
    return np.asarray(y, dtype=np.float32), exec_ns, profile_dir


def kernel(x, weight, gamma, beta):
    y, _ = run(x, weight, gamma, beta)
    return y
